# revision 30
# baseline (speedup 1.0000x reference)
"""Trainium2 Bass kernel for nn_CrossAttn (linear cross-attention, B=8 N=4096 C=1024 H=16).

Strategy:
  - Data-parallel over B across the 8 NeuronCores (batch-local math, no collectives).
  - Host pre-transposes activations to x^T [C, N] (C on partitions) and casts matmul
    operands to bf16; PSUM accumulation is fp32.
  - Linear-attention ctx via the Gram matrix: ctx_h = v_h^T k_h = Wv_h^T (x^T x) Wk_h.
    G = x^T x is accumulated once per stage (contraction over all N tokens) and shared
    by all 16 heads, replacing the [N, 2C] k/v materialization: 262k PE rows for G +
    66k for A = G @ Wk + 8k for ctx vs 524k + 33k on the direct path.
  - x is streamed from DRAM tile-by-tile, never SBUF-resident, so weight/x prefetch
    fully overlaps compute.  The host uploads x both channel-major (x^T, for the
    q GEMM / residual / output passes) and token-major (natural [N, C] layout,
    DMA'd straight into the self-stage Gram passes); the cross stages PE-transpose
    their streamed x' chunks instead, since x' only exists channel-major on device.
  - G accumulates in PSUM per 8-tile group (chunk-major: 2 live banks + 2 draining),
    groups summed into an SBUF fp32 accumulator by DVE; rounded to bf16 once.
  - ctx pairs (2 heads -> one 128x128 MM) accumulate over the 8 C-tiles directly in
    PSUM: 4 pairs cohabit a bank, so only the first matmul touching each bank uses
    start=True (clearing the bank's has_written); cohabitants' first matmuls use
    start=False, which overwrites where the bit is clear and accumulates after.
  - Softmax along the free axis; ctx transposed via PE into a block-diagonal 2-head
    bf16 tile; output product q @ ctx becomes (ctx_bd)^T @ q^T with K=128, N=512,
    residual fused into the mandatory PSUM->SBUF staging add (DVE), outputs staged
    bf16 so the host upcasts (halves output DMA).
  - Cross stage: q is the post-self activation itself; the o1 output pass shares the
    x1' DRAM stream with ctx1's Gram accumulation.
  - `iters` wraps the whole body (including DMAs) in a hardware For_i loop so test.py
    can measure steady-state per-iteration HW time as a slope between two counts.
"""

import os
import sys

sys.path.insert(0, "/opt/trn_rl_repo")

import numpy as np
import ml_dtypes

import concourse.bass as bass
import concourse.mybir as mybir
import concourse.tile as tile
from concourse import bacc
from concourse.masks import make_identity
from concourse.bass_utils import run_bass_kernel_spmd

B, N, C, H = 8, 4096, 1024, 16
D = C // H                 # 64
SCALE = D ** -0.5          # 0.125
P = 128                    # partitions
KT = C // P                # 8 contraction tiles
NT = N // P                # 32 token tiles
CH = N // 512              # 8 token chunks of 512
GRP = 8                    # token tiles per G PSUM group
NG = NT // GRP             # 4 groups
PAIRS = H // 2             # 8 head pairs
F32 = mybir.dt.float32
BF16 = mybir.dt.bfloat16

_CACHE = {}


def _build(iters: int = 1):
    nc = bacc.Bacc(None, target_bir_lowering=False)

    x1T_d = nc.dram_tensor("x1T", [C, N], BF16, kind="ExternalInput")
    x2T_d = nc.dram_tensor("x2T", [C, N], BF16, kind="ExternalInput")
    x1N_d = nc.dram_tensor("x1N", [N, C], BF16, kind="ExternalInput")
    x2N_d = nc.dram_tensor("x2N", [N, C], BF16, kind="ExternalInput")
    Wsqkv_d = nc.dram_tensor("Wsqkv", [C, 3 * C], BF16, kind="ExternalInput")
    Wkv1_d = nc.dram_tensor("Wkv1", [C, 2 * C], BF16, kind="ExternalInput")
    Wkv2_d = nc.dram_tensor("Wkv2", [C, 2 * C], BF16, kind="ExternalInput")
    o1T_d = nc.dram_tensor("o1T", [C, N], BF16, kind="ExternalOutput")
    o2T_d = nc.dram_tensor("o2T", [C, N], BF16, kind="ExternalOutput")
    x1p_scr = nc.dram_tensor("x1p_scratch", [C, N], BF16, kind="Internal")
    x2p_scr = nc.dram_tensor("x2p_scratch", [C, N], BF16, kind="Internal")

    # (kt*128 + p, n) -> [p, kt, n] view for per-partition-tile DMA
    x1T_r = x1T_d[:].rearrange("(t p) n -> p t n", p=P)
    x2T_r = x2T_d[:].rearrange("(t p) n -> p t n", p=P)
    x1N_r = x1N_d[:].rearrange("(t p) (kt q) -> p t kt q", p=P, q=P)
    x2N_r = x2N_d[:].rearrange("(t p) (kt q) -> p t kt q", p=P, q=P)
    Wsq_r = Wsqkv_d[:].rearrange("(t p) c -> p t c", p=P)
    Wkv1_r = Wkv1_d[:].rearrange("(t p) c -> p t c", p=P)
    Wkv2_r = Wkv2_d[:].rearrange("(t p) c -> p t c", p=P)
    o1T_r = o1T_d[:].rearrange("(t p) n -> p t n", p=P)
    o2T_r = o2T_d[:].rearrange("(t p) n -> p t n", p=P)
    x1p_r = x1p_scr[:].rearrange("(t p) n -> p t n", p=P)
    x2p_r = x2p_scr[:].rearrange("(t p) n -> p t n", p=P)

    with tile.TileContext(nc) as tc:
        with (
            tc.tile_pool(name="wts", bufs=1) as wts,
            tc.tile_pool(name="gacc", bufs=1) as gaccp,
            tc.tile_pool(name="gbf", bufs=1) as gbfp,
            tc.tile_pool(name="abf", bufs=1) as abfp,
            tc.tile_pool(name="xtok", bufs=2) as xtokp,
            tc.tile_pool(name="xch", bufs=4) as xchp,
            tc.tile_pool(name="qts", bufs=3) as qtsp,
            tc.tile_pool(name="ctxsb", bufs=2) as ctxsb,
            tc.tile_pool(name="smax", bufs=1) as smaxp,
            tc.tile_pool(name="stats", bufs=4) as stats,
            tc.tile_pool(name="outst", bufs=3) as outst,
            tc.tile_pool(name="singles", bufs=1) as singles,
            tc.tile_pool(name="ps_a", bufs=4, space="PSUM") as ps_a,    # 4 banks
            tc.tile_pool(name="ps_b", bufs=2, space="PSUM") as ps_b,    # 2 banks
            tc.tile_pool(name="ps_ctx", bufs=1, space="PSUM") as ps_ctx,  # 2 banks
        ):
            def _body():
                ident = singles.tile([P, P], F32)
                make_identity(nc, ident)
                identb = singles.tile([P, P], BF16)
                make_identity(nc, identb)

                def gram(x_r=None, extra_per_chunk=None, prefetch=None,
                         xtok_r=None, pre_per_chunk=None, after_row=None):
                    """G = x^T x accumulated over all NT token tiles.

                    Two sources for the token-major operand:
                    - xtok_r: DRAM view [p, t, kt, q] of x already token-major
                      (the host uploads x in its natural [N, C] layout); one
                      group DMA brings GRP token tiles in directly.
                    - x_r: DRAM view [p, kt, n] of x^T; streams CH chunks of
                      512 tokens and PE-transposes each 128-token sub-tile
                      (used for the cross stages, whose activations only exist
                      channel-major).

                    Accumulates only the upper triangle of chunk rows
                    (G[c1, c2] for c2 >= chunk base), chunk-major in PSUM (GRP
                    token tiles per accumulation group), summing groups into an
                    SBUF fp32 accumulator; the lower triangle is reconstructed
                    by symmetry with PE transposes after the bf16 rounding.

                    extra_per_chunk(ch, xch): emit extra work (the fused cross
                    output pass) consuming the same streamed x chunk.
                    prefetch: list of thunks, one issued per chunk (or per
                    group half), to slot weight-DMA pieces behind the x loads.
                    Returns G as [P, KT, C] bf16 (G[kt*128+p, c2]).
                    """
                    g32 = gaccp.tile([P, KT, C], F32, tag="gacc")
                    # per chunk row m: pieces of the upper-triangle width C-128m
                    pieces = {m: [(m * P + off, min(512, C - m * P - off))
                                  for off in range(0, C - m * P, 512)]
                              for m in range(KT)}
                    for g in range(NG):
                        xtok = xtokp.tile([P, GRP, KT, P], BF16, tag="xtok")
                        if xtok_r is not None:
                            nc.sync.dma_start(
                                out=xtok, in_=xtok_r[:, g * GRP:(g + 1) * GRP])
                            for half in range(2):
                                ch = g * 2 + half
                                if prefetch and ch < len(prefetch):
                                    prefetch[ch]()
                        else:
                            # GRP=8 token tiles arrive as 2 chunks of 512 tokens
                            for half in range(GRP // 4):
                                ch = g * 2 + half
                                xch = xchp.tile([P, KT, 512], BF16, tag="xch")
                                nc.sync.dma_start(
                                    out=xch, in_=x_r[:, :, ch * 512:(ch + 1) * 512])
                                if prefetch and ch < len(prefetch):
                                    prefetch[ch]()
                                if pre_per_chunk is not None:
                                    # softmax chains overlap the chunk's DMA
                                    pre_per_chunk(ch)
                                for t4 in range(4):
                                    tr_ps = ps_b.tile([P, 512], BF16, tag="psb")
                                    for kt in range(4):
                                        nc.tensor.transpose(
                                            tr_ps[:, kt * P:(kt + 1) * P],
                                            xch[:, kt, t4 * P:(t4 + 1) * P], identb)
                                    tr_ps2 = ps_b.tile([P, 512], BF16, tag="psb")
                                    for kt in range(4):
                                        nc.tensor.transpose(
                                            tr_ps2[:, kt * P:(kt + 1) * P],
                                            xch[:, 4 + kt, t4 * P:(t4 + 1) * P], identb)
                                    nt_l = half * 4 + t4
                                    nc.vector.tensor_copy(
                                        xtok[:, nt_l, 0:4, :].rearrange("p a b -> p (a b)"), tr_ps)
                                    nc.vector.tensor_copy(
                                        xtok[:, nt_l, 4:8, :].rearrange("p a b -> p (a b)"), tr_ps2)
                                if extra_per_chunk is not None:
                                    extra_per_chunk(ch, xch)
                        # chunk-major G accumulation: <=2 live banks per chunk
                        for m in range(KT):
                            gh = [ps_a.tile([P, 512], F32, tag="psa", name=f"gh{i}")
                                  for i in range(len(pieces[m]))]
                            for nt_l in range(GRP):
                                xv = xtok[:, nt_l, :, :].rearrange("p a b -> p (a b)")
                                for i, (c0, w) in enumerate(pieces[m]):
                                    nc.tensor.matmul(
                                        gh[i][:, 0:w],
                                        lhsT=xtok[:, nt_l, m, :],
                                        rhs=xv[:, c0:c0 + w],
                                        start=(nt_l == 0), stop=(nt_l == GRP - 1),
                                    )
                            for i, (c0, w) in enumerate(pieces[m]):
                                if g == 0:
                                    nc.vector.tensor_copy(
                                        g32[:, m, c0:c0 + w], gh[i][:, 0:w])
                                else:
                                    nc.vector.tensor_add(
                                        g32[:, m, c0:c0 + w],
                                        g32[:, m, c0:c0 + w], gh[i][:, 0:w])
                            if after_row is not None:
                                after_row(g, m)
                    gbf = gbfp.tile([P, KT, C], BF16, tag="gbf")
                    for m in range(KT):
                        nc.vector.tensor_copy(gbf[:, m, m * P:], g32[:, m, m * P:])
                    # lower triangle by symmetry: G[m-chunk, j] = G[j-chunk, m]^T;
                    # up to 4 transposes share one PSUM tile and one evacuation
                    for m in range(1, KT):
                        for j0 in range(0, m, 4):
                            nj = min(4, m - j0)
                            trg = ps_b.tile([P, 512], BF16, tag="psb")
                            for j in range(j0, j0 + nj):
                                nc.tensor.transpose(
                                    trg[:, (j - j0) * P:(j - j0 + 1) * P],
                                    gbf[:, j, m * P:(m + 1) * P], identb)
                            nc.vector.tensor_copy(
                                gbf[:, m, j0 * P:(j0 + nj) * P], trg[:, 0:nj * P])
                    return gbf

                def ctx_from_gram(gbf, W, kcol0, vcol0, sm_hook=None):
                    """ctx pairs (v^T k layout) from G: A = G @ Wk, ctx = Wv^T A.

                    Returns PSUM tile [P, PAIRS*128] fp32: pair p cols
                    [128p, 128p+128), rows = v-feature e, cols = k-feature d;
                    head 2p diag block at rows 0:64 cols +0:64, head 2p+1 at
                    rows 64:128 cols +64:128 (off-diag garbage, never read).
                    """
                    abf = abfp.tile([P, KT, C], BF16, tag="abf")
                    ctx_ps = ps_ctx.tile([P, PAIRS * P], F32, tag="ctx")
                    # jh-half-major: pairs 4jh..4jh+3 only need A columns of
                    # half jh, so their ctx matmuls (and softmax chains, via
                    # sm_hook) hide behind the other half's A matmuls
                    for jh in range(2):
                        for m in range(KT):
                            a_ps = ps_a.tile([P, 512], F32, tag="psa")
                            for kt in range(KT):
                                nc.tensor.matmul(
                                    a_ps,
                                    lhsT=gbf[:, kt, m * P:(m + 1) * P],
                                    rhs=W[:, kt, kcol0 + jh * 512: kcol0 + (jh + 1) * 512],
                                    start=(kt == 0), stop=(kt == KT - 1),
                                )
                            nc.vector.tensor_copy(abf[:, m, jh * 512:(jh + 1) * 512], a_ps)
                        for kt in range(KT):
                            for p in range(4 * jh, 4 * jh + 4):
                                nc.tensor.matmul(
                                    ctx_ps[:, p * P:(p + 1) * P],
                                    lhsT=W[:, kt, vcol0 + p * P: vcol0 + (p + 1) * P],
                                    rhs=abf[:, kt, p * P:(p + 1) * P],
                                    # 4 pairs cohabit each PSUM bank: the bank's
                                    # first matmul clears it (start=True); the
                                    # other pairs' first matmuls overwrite via
                                    # the cleared has_written bits; only the
                                    # bank's very last matmul ends the group.
                                    start=(kt == 0 and p % 4 == 0),
                                    stop=(kt == KT - 1 and p % 4 == 3),
                                )
                        if sm_hook is not None:
                            for p in range(4 * jh, 4 * jh + 4):
                                sm_hook(ctx_ps, p)
                    return ctx_ps

                # off-diag blocks of every softmax slice stay zero; exp only
                # ever rewrites the diag blocks, so one memset serves all
                # stages and loop iterations
                Sbig = smaxp.tile([P, PAIRS * P], F32, tag="smax")
                nc.vector.memset(Sbig, 0.0)

                def softmax_pair(ctx_ps, p, ctx_bd):
                    """Softmax over d (free axis) of the two diag blocks of pair p,
                    then PE-transpose into slice p of the block-diag bf16 ctx tile.

                    The max-subtraction is required on HW: the Act engine's exp
                    is table-based and inputs must be kept <= 0 (removing it
                    passes the interpreter but produces NaN on hardware)."""
                    S = Sbig[:, p * P:(p + 1) * P]
                    for r0 in (0, 64):
                        blk = ctx_ps[r0:r0 + 64, p * P + r0: p * P + r0 + 64]
                        mx = stats.tile([P, 1], F32, tag="mx")
                        nc.vector.reduce_max(mx[r0:r0 + 64], blk, axis=mybir.AxisListType.X)
                        ng = stats.tile([P, 1], F32, tag="ng")
                        nc.scalar.mul(ng[r0:r0 + 64], mx[r0:r0 + 64], -SCALE)
                        se = stats.tile([P, 1], F32, tag="se")
                        nc.scalar.activation(
                            S[r0:r0 + 64, r0:r0 + 64], blk,
                            mybir.ActivationFunctionType.Exp,
                            bias=ng[r0:r0 + 64], scale=SCALE,
                            accum_out=se[r0:r0 + 64],
                        )
                        rv = stats.tile([P, 1], F32, tag="rv")
                        nc.vector.reciprocal(rv[r0:r0 + 64], se[r0:r0 + 64])
                        nc.vector.tensor_scalar_mul(
                            S[r0:r0 + 64, r0:r0 + 64], S[r0:r0 + 64, r0:r0 + 64],
                            rv[r0:r0 + 64],
                        )
                    tr_ps = ps_b.tile([P, P], F32, tag="psb")
                    nc.tensor.transpose(tr_ps, S, ident)
                    nc.vector.tensor_copy(ctx_bd[:, p, :], tr_ps)

                def self_stage(x_r, xn_r, Wsq, xp_out_r, prefetch=None):
                    """One self-attention branch: x' = merge(q @ ctx) + x, spilled
                    bf16-transposed to DRAM via xp_out_r."""
                    gbf = gram(xtok_r=xn_r, prefetch=prefetch)
                    ctx_bd = ctxsb.tile([P, PAIRS, P], BF16, tag="ctx_bd")
                    ctx_from_gram(gbf, Wsq, kcol0=C, vcol0=2 * C,
                                  sm_hook=lambda cps, p: softmax_pair(cps, p, ctx_bd))
                    for ch in range(CH):
                        xch = xchp.tile([P, KT, 512], BF16, tag="xch")
                        nc.sync.dma_start(
                            out=xch, in_=x_r[:, :, ch * 512:(ch + 1) * 512])
                        for p in range(PAIRS):
                            qt_ps = ps_a.tile([P, 512], F32, tag="psa")
                            for kt in range(KT):
                                nc.tensor.matmul(
                                    qt_ps,
                                    lhsT=Wsq[:, kt, p * P:(p + 1) * P],
                                    rhs=xch[:, kt, :],
                                    start=(kt == 0), stop=(kt == KT - 1),
                                )
                            qts = qtsp.tile([P, 512], BF16, tag="qts")
                            nc.vector.tensor_copy(qts, qt_ps)
                            out_ps = ps_b.tile([P, 512], F32, tag="psb")
                            nc.tensor.matmul(out_ps, lhsT=ctx_bd[:, p, :], rhs=qts,
                                             start=True, stop=True)
                            stg = outst.tile([P, 512], BF16, tag="stg")
                            nc.vector.tensor_add(stg, out_ps, xch[:, p, :])
                            nc.sync.dma_start(
                                out=xp_out_r[:, p, ch * 512:(ch + 1) * 512], in_=stg)
                    return ctx_bd

                def cross_out_chunk(o_r, ctx_bd, ch, xch):
                    """o chunk = merge(q @ ctx) + q_residual for one 512-token
                    chunk of the streamed post-self activation."""
                    for p in range(PAIRS):
                        out_ps = ps_b.tile([P, 512], F32, tag="psb")
                        nc.tensor.matmul(out_ps, lhsT=ctx_bd[:, p, :],
                                         rhs=xch[:, p, :], start=True, stop=True)
                        stg = outst.tile([P, 512], BF16, tag="stg")
                        nc.vector.tensor_add(stg, out_ps, xch[:, p, :])
                        nc.sync.dma_start(
                            out=o_r[:, p, ch * 512:(ch + 1) * 512], in_=stg)

                # ---- weights: pieces slotted behind self-1's x chunk loads ----
                Wsq = wts.tile([P, KT, 3 * C], BF16, tag="w")

                def wsq_piece(i):
                    nc.sync.dma_start(out=Wsq[:, :, i * 512:(i + 1) * 512],
                                      in_=Wsq_r[:, :, i * 512:(i + 1) * 512])

                # ---- self stages ----
                self_stage(x1T_r, x1N_r, Wsq, x1p_r,
                           prefetch=[lambda i=i: wsq_piece(i) for i in range(6)])
                self_stage(x2T_r, x2N_r, Wsq, x2p_r)

                # shares the "w" slot: each piece DMA waits for Wsq's last read
                # (end of self-2 pass 2), then overlaps the weight-free g2 pass
                Wkv = wts.tile([P, KT, 4 * C], BF16, tag="w")

                def wkv_piece(i):
                    src_r, off = (Wkv2_r, 0) if i < 4 else (Wkv1_r, 2 * C)
                    j = (i % 4) * 512
                    nc.sync.dma_start(out=Wkv[:, :, off + j: off + j + 512],
                                      in_=src_r[:, :, j:j + 512])

                # ---- cross: ctx2 from x2' ----
                g2 = gram(x_r=x2p_r,
                          prefetch=[lambda i=i: wkv_piece(i) for i in range(8)])
                ctx2_ps = ctx_from_gram(g2, Wkv, kcol0=0, vcol0=C)
                ctx2_bd = ctxsb.tile([P, PAIRS, P], BF16, tag="ctx_bd")

                # ---- cross: o1 output pass fused with ctx1's Gram stream.
                # ctx2's softmax pairs are emitted inside the stream's first
                # two chunks (their chains hide behind the chunk DMAs), and
                # the o1 matmuls are delayed two chunks so every ctx2_bd
                # write precedes its first PE consumer in program order. ----
                pending1 = []

                def sm2_row(g, m):
                    # each ctx2 softmax chain hides behind the next G chunk-row
                    if g == 0:
                        softmax_pair(ctx2_ps, m, ctx2_bd)

                def extra1(ch, xch):
                    pending1.append((ch, xch))
                    if len(pending1) > 2:
                        c, xc = pending1.pop(0)
                        cross_out_chunk(o1T_r, ctx2_bd, c, xc)

                g1 = gram(x_r=x1p_r, after_row=sm2_row, extra_per_chunk=extra1)
                for c, xc in pending1:
                    cross_out_chunk(o1T_r, ctx2_bd, c, xc)

                # o2's first chunks prefetch while A1/ctx1 compute
                o2pre = []
                for ch in range(2):
                    xch = xchp.tile([P, KT, 512], BF16, tag="xch")
                    nc.sync.dma_start(out=xch, in_=x2p_r[:, :, ch * 512:(ch + 1) * 512])
                    o2pre.append(xch)

                ctx1_bd = ctxsb.tile([P, PAIRS, P], BF16, tag="ctx_bd")
                ctx_from_gram(g1, Wkv, kcol0=2 * C, vcol0=3 * C,
                              sm_hook=lambda cps, p: softmax_pair(cps, p, ctx1_bd))

                # ---- cross: o2 output pass ----
                for ch in range(CH):
                    if ch < 2:
                        xch = o2pre[ch]
                    else:
                        xch = xchp.tile([P, KT, 512], BF16, tag="xch")
                        nc.sync.dma_start(out=xch, in_=x2p_r[:, :, ch * 512:(ch + 1) * 512])
                    cross_out_chunk(o2T_r, ctx1_bd, ch, xch)

            if iters > 1:
                with tc.For_i(0, iters):
                    _body()
            else:
                _body()

    nc.finalize()
    return nc


def _get_nc():
    if "nc" not in _CACHE:
        _CACHE["nc"] = _build()
    return _CACHE["nc"]


def _make_in_maps(np_inputs):
    x1 = np.asarray(np_inputs["x1"], dtype=np.float32)
    x2 = np.asarray(np_inputs["x2"], dtype=np.float32)
    Wsq_b = np.ascontiguousarray(np.asarray(np_inputs["Wsqkv1"], np.float32)).astype(ml_dtypes.bfloat16)
    Wkv1_b = np.ascontiguousarray(np.asarray(np_inputs["Wkv1"], np.float32)).astype(ml_dtypes.bfloat16)
    Wkv2_b = np.ascontiguousarray(np.asarray(np_inputs["Wkv2"], np.float32)).astype(ml_dtypes.bfloat16)
    return [{
        "x1T": np.ascontiguousarray(x1[b].T).astype(ml_dtypes.bfloat16),
        "x2T": np.ascontiguousarray(x2[b].T).astype(ml_dtypes.bfloat16),
        "x1N": np.ascontiguousarray(x1[b]).astype(ml_dtypes.bfloat16),
        "x2N": np.ascontiguousarray(x2[b]).astype(ml_dtypes.bfloat16),
        "Wsqkv": Wsq_b,
        "Wkv1": Wkv1_b,
        "Wkv2": Wkv2_b,
    } for b in range(B)]


def _unpack_results(results):
    o1 = np.stack([np.asarray(results[b]["o1T"]).astype(np.float32).T for b in range(B)])
    o2 = np.stack([np.asarray(results[b]["o2T"]).astype(np.float32).T for b in range(B)])
    return o1, o2


def kernel(x1, x2, Wsqkv1, Wkv1, Wkv2, num_heads=16, selfattn=1, **_unused):
    in_maps = _make_in_maps(dict(x1=x1, x2=x2, Wsqkv1=Wsqkv1, Wkv1=Wkv1, Wkv2=Wkv2))
    nc = _get_nc()
    res = run_bass_kernel_spmd(nc, in_maps, core_ids=list(range(B)),
                               trace=bool(int(os.environ.get("KERNEL_TRACE", "0"))))
    _CACHE["last_result"] = res
    return _unpack_results(res.results)


# revision 31
# speedup vs baseline: 1.0205x; 1.0205x over previous
"""Trainium2 Bass kernel for nn_CrossAttn (linear cross-attention, B=8 N=4096 C=1024 H=16).

Strategy:
  - Data-parallel over B across the 8 NeuronCores (batch-local math, no collectives).
  - Host pre-transposes activations to x^T [C, N] (C on partitions) and casts matmul
    operands to bf16; PSUM accumulation is fp32.
  - Linear-attention ctx via the Gram matrix: ctx_h = v_h^T k_h = Wv_h^T (x^T x) Wk_h.
    G = x^T x is accumulated once per stage (contraction over all N tokens) and shared
    by all 16 heads, replacing the [N, 2C] k/v materialization: 262k PE rows for G +
    66k for A = G @ Wk + 8k for ctx vs 524k + 33k on the direct path.
  - x is streamed from DRAM tile-by-tile, never SBUF-resident, so weight/x prefetch
    fully overlaps compute.  The host uploads x both channel-major (x^T, for the
    q GEMM / residual / output passes) and token-major (natural [N, C] layout,
    DMA'd straight into the self-stage Gram passes); the cross stages PE-transpose
    their streamed x' chunks instead, since x' only exists channel-major on device.
  - G accumulates in PSUM per 8-tile group (chunk-major: 2 live banks + 2 draining),
    groups summed into an SBUF fp32 accumulator by DVE; rounded to bf16 once.
  - ctx pairs (2 heads -> one 128x128 MM) accumulate over the 8 C-tiles directly in
    PSUM: 4 pairs cohabit a bank, so only the first matmul touching each bank uses
    start=True (clearing the bank's has_written); cohabitants' first matmuls use
    start=False, which overwrites where the bit is clear and accumulates after.
  - Softmax along the free axis; ctx transposed via PE into a block-diagonal 2-head
    bf16 tile; output product q @ ctx becomes (ctx_bd)^T @ q^T with K=128, N=512,
    residual fused into the mandatory PSUM->SBUF staging add (DVE), outputs staged
    bf16 so the host upcasts (halves output DMA).
  - Cross stage: q is the post-self activation itself; the o1 output pass shares the
    x1' DRAM stream with ctx1's Gram accumulation.
  - `iters` wraps the whole body (including DMAs) in a hardware For_i loop so test.py
    can measure steady-state per-iteration HW time as a slope between two counts.
"""

import os
import sys

sys.path.insert(0, "/opt/trn_rl_repo")

import numpy as np
import ml_dtypes

import concourse.bass as bass
import concourse.mybir as mybir
import concourse.tile as tile
from concourse import bacc
from concourse.masks import make_identity
from concourse.bass_utils import run_bass_kernel_spmd

B, N, C, H = 8, 4096, 1024, 16
D = C // H                 # 64
SCALE = D ** -0.5          # 0.125
P = 128                    # partitions
KT = C // P                # 8 contraction tiles
NT = N // P                # 32 token tiles
CH = N // 512              # 8 token chunks of 512
GRP = 8                    # token tiles per G PSUM group
NG = NT // GRP             # 4 groups
PAIRS = H // 2             # 8 head pairs
F32 = mybir.dt.float32
BF16 = mybir.dt.bfloat16

_CACHE = {}


def _build(iters: int = 1):
    nc = bacc.Bacc(None, target_bir_lowering=False)

    x1T_d = nc.dram_tensor("x1T", [C, N], BF16, kind="ExternalInput")
    x2T_d = nc.dram_tensor("x2T", [C, N], BF16, kind="ExternalInput")
    x1N_d = nc.dram_tensor("x1N", [N, C], BF16, kind="ExternalInput")
    x2N_d = nc.dram_tensor("x2N", [N, C], BF16, kind="ExternalInput")
    Wsqkv_d = nc.dram_tensor("Wsqkv", [C, 3 * C], BF16, kind="ExternalInput")
    Wkv1_d = nc.dram_tensor("Wkv1", [C, 2 * C], BF16, kind="ExternalInput")
    Wkv2_d = nc.dram_tensor("Wkv2", [C, 2 * C], BF16, kind="ExternalInput")
    o1T_d = nc.dram_tensor("o1T", [C, N], BF16, kind="ExternalOutput")
    o2T_d = nc.dram_tensor("o2T", [C, N], BF16, kind="ExternalOutput")
    x1p_scr = nc.dram_tensor("x1p_scratch", [C, N], BF16, kind="Internal")
    x2p_scr = nc.dram_tensor("x2p_scratch", [C, N], BF16, kind="Internal")

    # (kt*128 + p, n) -> [p, kt, n] view for per-partition-tile DMA
    x1T_r = x1T_d[:].rearrange("(t p) n -> p t n", p=P)
    x2T_r = x2T_d[:].rearrange("(t p) n -> p t n", p=P)
    x1N_r = x1N_d[:].rearrange("(t p) (kt q) -> p t kt q", p=P, q=P)
    x2N_r = x2N_d[:].rearrange("(t p) (kt q) -> p t kt q", p=P, q=P)
    Wsq_r = Wsqkv_d[:].rearrange("(t p) c -> p t c", p=P)
    Wkv1_r = Wkv1_d[:].rearrange("(t p) c -> p t c", p=P)
    Wkv2_r = Wkv2_d[:].rearrange("(t p) c -> p t c", p=P)
    o1T_r = o1T_d[:].rearrange("(t p) n -> p t n", p=P)
    o2T_r = o2T_d[:].rearrange("(t p) n -> p t n", p=P)
    x1p_r = x1p_scr[:].rearrange("(t p) n -> p t n", p=P)
    x2p_r = x2p_scr[:].rearrange("(t p) n -> p t n", p=P)

    with tile.TileContext(nc) as tc:
        with (
            tc.tile_pool(name="wts", bufs=1) as wts,
            tc.tile_pool(name="gacc", bufs=1) as gaccp,
            tc.tile_pool(name="gbf", bufs=1) as gbfp,
            tc.tile_pool(name="abf", bufs=1) as abfp,
            tc.tile_pool(name="xtok", bufs=2) as xtokp,
            tc.tile_pool(name="xch", bufs=4) as xchp,
            tc.tile_pool(name="ctxsb", bufs=2) as ctxsb,
            tc.tile_pool(name="smax", bufs=1) as smaxp,
            tc.tile_pool(name="stats", bufs=4) as stats,
            tc.tile_pool(name="outst", bufs=3) as outst,
            tc.tile_pool(name="singles", bufs=1) as singles,
            tc.tile_pool(name="ps_a", bufs=4, space="PSUM") as ps_a,    # 4 banks
            tc.tile_pool(name="ps_b", bufs=2, space="PSUM") as ps_b,    # 2 banks
            tc.tile_pool(name="ps_ctx", bufs=1, space="PSUM") as ps_ctx,  # 2 banks
        ):
            def _body():
                ident = singles.tile([P, P], F32)
                make_identity(nc, ident)
                identb = singles.tile([P, P], BF16)
                make_identity(nc, identb)

                def gram(x_r=None, extra_per_chunk=None, prefetch=None,
                         xtok_r=None, pre_per_chunk=None, after_row=None):
                    """G = x^T x accumulated over all NT token tiles.

                    Two sources for the token-major operand:
                    - xtok_r: DRAM view [p, t, kt, q] of x already token-major
                      (the host uploads x in its natural [N, C] layout); one
                      group DMA brings GRP token tiles in directly.
                    - x_r: DRAM view [p, kt, n] of x^T; streams CH chunks of
                      512 tokens and PE-transposes each 128-token sub-tile
                      (used for the cross stages, whose activations only exist
                      channel-major).

                    Accumulates only the upper triangle of chunk rows
                    (G[c1, c2] for c2 >= chunk base), chunk-major in PSUM (GRP
                    token tiles per accumulation group), summing groups into an
                    SBUF fp32 accumulator; the lower triangle is reconstructed
                    by symmetry with PE transposes after the bf16 rounding.

                    extra_per_chunk(ch, xch): emit extra work (the fused cross
                    output pass) consuming the same streamed x chunk.
                    prefetch: list of thunks, one issued per chunk (or per
                    group half), to slot weight-DMA pieces behind the x loads.
                    Returns G as [P, KT, C] bf16 (G[kt*128+p, c2]).
                    """
                    g32 = gaccp.tile([P, KT, C], F32, tag="gacc")
                    # per chunk row m: pieces of the upper-triangle width C-128m
                    pieces = {m: [(m * P + off, min(512, C - m * P - off))
                                  for off in range(0, C - m * P, 512)]
                              for m in range(KT)}
                    for g in range(NG):
                        xtok = xtokp.tile([P, GRP, KT, P], BF16, tag="xtok")
                        if xtok_r is not None:
                            nc.sync.dma_start(
                                out=xtok, in_=xtok_r[:, g * GRP:(g + 1) * GRP])
                            for half in range(2):
                                ch = g * 2 + half
                                if prefetch and ch < len(prefetch):
                                    prefetch[ch]()
                        else:
                            # GRP=8 token tiles arrive as 2 chunks of 512 tokens
                            for half in range(GRP // 4):
                                ch = g * 2 + half
                                xch = xchp.tile([P, KT, 512], BF16, tag="xch")
                                nc.sync.dma_start(
                                    out=xch, in_=x_r[:, :, ch * 512:(ch + 1) * 512])
                                if prefetch and ch < len(prefetch):
                                    prefetch[ch]()
                                if pre_per_chunk is not None:
                                    # softmax chains overlap the chunk's DMA
                                    pre_per_chunk(ch)
                                for t4 in range(4):
                                    tr_ps = ps_b.tile([P, 512], BF16, tag="psb")
                                    for kt in range(4):
                                        nc.tensor.transpose(
                                            tr_ps[:, kt * P:(kt + 1) * P],
                                            xch[:, kt, t4 * P:(t4 + 1) * P], identb)
                                    tr_ps2 = ps_b.tile([P, 512], BF16, tag="psb")
                                    for kt in range(4):
                                        nc.tensor.transpose(
                                            tr_ps2[:, kt * P:(kt + 1) * P],
                                            xch[:, 4 + kt, t4 * P:(t4 + 1) * P], identb)
                                    nt_l = half * 4 + t4
                                    nc.vector.tensor_copy(
                                        xtok[:, nt_l, 0:4, :].rearrange("p a b -> p (a b)"), tr_ps)
                                    nc.vector.tensor_copy(
                                        xtok[:, nt_l, 4:8, :].rearrange("p a b -> p (a b)"), tr_ps2)
                                if extra_per_chunk is not None:
                                    extra_per_chunk(ch, xch)
                        # chunk-major G accumulation: <=2 live banks per chunk
                        for m in range(KT):
                            gh = [ps_a.tile([P, 512], F32, tag="psa", name=f"gh{i}")
                                  for i in range(len(pieces[m]))]
                            for nt_l in range(GRP):
                                xv = xtok[:, nt_l, :, :].rearrange("p a b -> p (a b)")
                                for i, (c0, w) in enumerate(pieces[m]):
                                    nc.tensor.matmul(
                                        gh[i][:, 0:w],
                                        lhsT=xtok[:, nt_l, m, :],
                                        rhs=xv[:, c0:c0 + w],
                                        start=(nt_l == 0), stop=(nt_l == GRP - 1),
                                    )
                            for i, (c0, w) in enumerate(pieces[m]):
                                if g == 0:
                                    nc.vector.tensor_copy(
                                        g32[:, m, c0:c0 + w], gh[i][:, 0:w])
                                else:
                                    nc.vector.tensor_add(
                                        g32[:, m, c0:c0 + w],
                                        g32[:, m, c0:c0 + w], gh[i][:, 0:w])
                            if after_row is not None:
                                after_row(g, m)
                    gbf = gbfp.tile([P, KT, C], BF16, tag="gbf")
                    for m in range(KT):
                        nc.vector.tensor_copy(gbf[:, m, m * P:], g32[:, m, m * P:])
                    # lower triangle by symmetry: G[m-chunk, j] = G[j-chunk, m]^T;
                    # up to 4 transposes share one PSUM tile and one evacuation
                    for m in range(1, KT):
                        for j0 in range(0, m, 4):
                            nj = min(4, m - j0)
                            trg = ps_b.tile([P, 512], BF16, tag="psb")
                            for j in range(j0, j0 + nj):
                                nc.tensor.transpose(
                                    trg[:, (j - j0) * P:(j - j0 + 1) * P],
                                    gbf[:, j, m * P:(m + 1) * P], identb)
                            nc.vector.tensor_copy(
                                gbf[:, m, j0 * P:(j0 + nj) * P], trg[:, 0:nj * P])
                    return gbf

                def ctx_from_gram(gbf, W, kcol0, vcol0, sm_hook=None):
                    """ctx pairs (v^T k layout) from G: A = G @ Wk, ctx = Wv^T A.

                    Returns PSUM tile [P, PAIRS*128] fp32: pair p cols
                    [128p, 128p+128), rows = v-feature e, cols = k-feature d;
                    head 2p diag block at rows 0:64 cols +0:64, head 2p+1 at
                    rows 64:128 cols +64:128 (off-diag garbage, never read).
                    """
                    abf = abfp.tile([P, KT, C], BF16, tag="abf")
                    ctx_ps = ps_ctx.tile([P, PAIRS * P], F32, tag="ctx")
                    # jh-half-major: pairs 4jh..4jh+3 only need A columns of
                    # half jh, so their ctx matmuls (and softmax chains, via
                    # sm_hook) hide behind the other half's A matmuls
                    for jh in range(2):
                        for m in range(KT):
                            a_ps = ps_a.tile([P, 512], F32, tag="psa")
                            for kt in range(KT):
                                nc.tensor.matmul(
                                    a_ps,
                                    lhsT=gbf[:, kt, m * P:(m + 1) * P],
                                    rhs=W[:, kt, kcol0 + jh * 512: kcol0 + (jh + 1) * 512],
                                    start=(kt == 0), stop=(kt == KT - 1),
                                )
                            nc.vector.tensor_copy(abf[:, m, jh * 512:(jh + 1) * 512], a_ps)
                        for kt in range(KT):
                            for p in range(4 * jh, 4 * jh + 4):
                                nc.tensor.matmul(
                                    ctx_ps[:, p * P:(p + 1) * P],
                                    lhsT=W[:, kt, vcol0 + p * P: vcol0 + (p + 1) * P],
                                    rhs=abf[:, kt, p * P:(p + 1) * P],
                                    # 4 pairs cohabit each PSUM bank: the bank's
                                    # first matmul clears it (start=True); the
                                    # other pairs' first matmuls overwrite via
                                    # the cleared has_written bits; only the
                                    # bank's very last matmul ends the group.
                                    start=(kt == 0 and p % 4 == 0),
                                    stop=(kt == KT - 1 and p % 4 == 3),
                                )
                        if sm_hook is not None:
                            for p in range(4 * jh, 4 * jh + 4):
                                sm_hook(ctx_ps, p)
                    return ctx_ps

                # off-diag blocks of every softmax slice stay zero; exp only
                # ever rewrites the diag blocks, so one memset serves all
                # stages and loop iterations
                Sbig = smaxp.tile([P, PAIRS * P], F32, tag="smax")
                nc.vector.memset(Sbig, 0.0)

                def softmax_pair(ctx_ps, p, ctx_bd):
                    """Softmax over d (free axis) of the two diag blocks of pair p,
                    then PE-transpose into slice p of the block-diag bf16 ctx tile.

                    The max-subtraction is required on HW: the Act engine's exp
                    is table-based and inputs must be kept <= 0 (removing it
                    passes the interpreter but produces NaN on hardware)."""
                    S = Sbig[:, p * P:(p + 1) * P]
                    for r0 in (0, 64):
                        blk = ctx_ps[r0:r0 + 64, p * P + r0: p * P + r0 + 64]
                        mx = stats.tile([P, 1], F32, tag="mx")
                        nc.vector.reduce_max(mx[r0:r0 + 64], blk, axis=mybir.AxisListType.X)
                        ng = stats.tile([P, 1], F32, tag="ng")
                        nc.scalar.mul(ng[r0:r0 + 64], mx[r0:r0 + 64], -SCALE)
                        se = stats.tile([P, 1], F32, tag="se")
                        nc.scalar.activation(
                            S[r0:r0 + 64, r0:r0 + 64], blk,
                            mybir.ActivationFunctionType.Exp,
                            bias=ng[r0:r0 + 64], scale=SCALE,
                            accum_out=se[r0:r0 + 64],
                        )
                        rv = stats.tile([P, 1], F32, tag="rv")
                        nc.vector.reciprocal(rv[r0:r0 + 64], se[r0:r0 + 64])
                        nc.vector.tensor_scalar_mul(
                            S[r0:r0 + 64, r0:r0 + 64], S[r0:r0 + 64, r0:r0 + 64],
                            rv[r0:r0 + 64],
                        )
                    tr_ps = ps_b.tile([P, P], F32, tag="psb")
                    nc.tensor.transpose(tr_ps, S, ident)
                    nc.vector.tensor_copy(ctx_bd[:, p, :], tr_ps)

                def self_stage(x_r, xn_r, Wsq, xp_out_r, prefetch=None):
                    """One self-attention branch via weight folding: x' = x @ Wtil
                    + x with Wtil = Wq @ blockdiag(ctx).  WqT is built right after
                    the Gram pass (aliasing the then-dead gacc slot); each pair's
                    Wtil matmuls are emitted inside the softmax hook so their
                    ctx_bd dependency hides behind the other half's A matmuls."""
                    gbf = gram(xtok_r=xn_r, prefetch=prefetch)
                    # WqT and Wtil pack into the 32KB gacc slot (dead after gram)
                    wq_wt = gaccp.tile([P, 2, KT, PAIRS * P], BF16, tag="gacc")
                    for p in range(PAIRS):
                        for ktc in range(0, KT, 4):
                            trw = ps_b.tile([P, 512], BF16, tag="psb")
                            for k4 in range(4):
                                nc.tensor.transpose(
                                    trw[:, k4 * P:(k4 + 1) * P],
                                    Wsq[:, ktc + k4, p * P:(p + 1) * P], identb)
                            nc.vector.tensor_copy(
                                wq_wt[:, 0, ktc:ktc + 4, p * P:(p + 1) * P],
                                trw.rearrange("p (a b) -> p a b", a=4))
                    ctx_bd = ctxsb.tile([P, PAIRS, P], BF16, tag="ctx_bd")

                    def sm_and_wtil(cps, p):
                        softmax_pair(cps, p, ctx_bd)
                        for m in range(KT):
                            wt_ps = ps_b.tile([P, P], F32, tag="psb")
                            nc.tensor.matmul(wt_ps, lhsT=wq_wt[:, 0, m, p * P:(p + 1) * P],
                                             rhs=ctx_bd[:, p, :], start=True, stop=True)
                            nc.vector.tensor_copy(
                                wq_wt[:, 1, m, p * P:(p + 1) * P], wt_ps)

                    ctx_from_gram(gbf, Wsq, kcol0=C, vcol0=2 * C,
                                  sm_hook=sm_and_wtil)
                    for ch in range(CH):
                        xch = xchp.tile([P, KT, 512], BF16, tag="xch")
                        nc.sync.dma_start(
                            out=xch, in_=x_r[:, :, ch * 512:(ch + 1) * 512])
                        for jt in range(KT):
                            o_ps = ps_a.tile([P, 512], F32, tag="psa")
                            for kt in range(KT):
                                nc.tensor.matmul(
                                    o_ps,
                                    lhsT=wq_wt[:, 1, kt, jt * P:(jt + 1) * P],
                                    rhs=xch[:, kt, :],
                                    start=(kt == 0), stop=(kt == KT - 1),
                                )
                            stg = outst.tile([P, 512], BF16, tag="stg")
                            nc.vector.tensor_add(stg, o_ps, xch[:, jt, :])
                            nc.sync.dma_start(
                                out=xp_out_r[:, jt, ch * 512:(ch + 1) * 512], in_=stg)
                    return ctx_bd

                def cross_out_chunk(o_r, ctx_bd, ch, xch):
                    """o chunk = merge(q @ ctx) + q_residual for one 512-token
                    chunk of the streamed post-self activation."""
                    for p in range(PAIRS):
                        out_ps = ps_b.tile([P, 512], F32, tag="psb")
                        nc.tensor.matmul(out_ps, lhsT=ctx_bd[:, p, :],
                                         rhs=xch[:, p, :], start=True, stop=True)
                        stg = outst.tile([P, 512], BF16, tag="stg")
                        nc.vector.tensor_add(stg, out_ps, xch[:, p, :])
                        nc.sync.dma_start(
                            out=o_r[:, p, ch * 512:(ch + 1) * 512], in_=stg)

                # ---- weights: pieces slotted behind self-1's x chunk loads ----
                Wsq = wts.tile([P, KT, 3 * C], BF16, tag="w")

                def wsq_piece(i):
                    nc.sync.dma_start(out=Wsq[:, :, i * 512:(i + 1) * 512],
                                      in_=Wsq_r[:, :, i * 512:(i + 1) * 512])

                # ---- self stages ----
                self_stage(x1T_r, x1N_r, Wsq, x1p_r,
                           prefetch=[lambda i=i: wsq_piece(i) for i in range(6)])
                self_stage(x2T_r, x2N_r, Wsq, x2p_r)

                # shares the "w" slot: each piece DMA waits for Wsq's last read
                # (end of self-2 pass 2), then overlaps the weight-free g2 pass
                Wkv = wts.tile([P, KT, 4 * C], BF16, tag="w")

                def wkv_piece(i):
                    src_r, off = (Wkv2_r, 0) if i < 4 else (Wkv1_r, 2 * C)
                    j = (i % 4) * 512
                    nc.sync.dma_start(out=Wkv[:, :, off + j: off + j + 512],
                                      in_=src_r[:, :, j:j + 512])

                # ---- cross: ctx2 from x2' ----
                g2 = gram(x_r=x2p_r,
                          prefetch=[lambda i=i: wkv_piece(i) for i in range(8)])
                ctx2_ps = ctx_from_gram(g2, Wkv, kcol0=0, vcol0=C)
                ctx2_bd = ctxsb.tile([P, PAIRS, P], BF16, tag="ctx_bd")

                # ---- cross: o1 output pass fused with ctx1's Gram stream.
                # ctx2's softmax pairs are emitted inside the stream's first
                # two chunks (their chains hide behind the chunk DMAs), and
                # the o1 matmuls are delayed two chunks so every ctx2_bd
                # write precedes its first PE consumer in program order. ----
                pending1 = []

                def sm2_row(g, m):
                    # each ctx2 softmax chain hides behind the next G chunk-row
                    if g == 0:
                        softmax_pair(ctx2_ps, m, ctx2_bd)

                def extra1(ch, xch):
                    pending1.append((ch, xch))
                    if len(pending1) > 2:
                        c, xc = pending1.pop(0)
                        cross_out_chunk(o1T_r, ctx2_bd, c, xc)

                g1 = gram(x_r=x1p_r, after_row=sm2_row, extra_per_chunk=extra1)
                for c, xc in pending1:
                    cross_out_chunk(o1T_r, ctx2_bd, c, xc)

                # o2's first chunks prefetch while A1/ctx1 compute
                o2pre = []
                for ch in range(2):
                    xch = xchp.tile([P, KT, 512], BF16, tag="xch")
                    nc.sync.dma_start(out=xch, in_=x2p_r[:, :, ch * 512:(ch + 1) * 512])
                    o2pre.append(xch)

                ctx1_bd = ctxsb.tile([P, PAIRS, P], BF16, tag="ctx_bd")
                ctx_from_gram(g1, Wkv, kcol0=2 * C, vcol0=3 * C,
                              sm_hook=lambda cps, p: softmax_pair(cps, p, ctx1_bd))

                # ---- cross: o2 output pass ----
                for ch in range(CH):
                    if ch < 2:
                        xch = o2pre[ch]
                    else:
                        xch = xchp.tile([P, KT, 512], BF16, tag="xch")
                        nc.sync.dma_start(out=xch, in_=x2p_r[:, :, ch * 512:(ch + 1) * 512])
                    cross_out_chunk(o2T_r, ctx1_bd, ch, xch)

            if iters > 1:
                with tc.For_i(0, iters):
                    _body()
            else:
                _body()

    nc.finalize()
    return nc


def _get_nc():
    if "nc" not in _CACHE:
        _CACHE["nc"] = _build()
    return _CACHE["nc"]


def _make_in_maps(np_inputs):
    x1 = np.asarray(np_inputs["x1"], dtype=np.float32)
    x2 = np.asarray(np_inputs["x2"], dtype=np.float32)
    Wsq_b = np.ascontiguousarray(np.asarray(np_inputs["Wsqkv1"], np.float32)).astype(ml_dtypes.bfloat16)
    Wkv1_b = np.ascontiguousarray(np.asarray(np_inputs["Wkv1"], np.float32)).astype(ml_dtypes.bfloat16)
    Wkv2_b = np.ascontiguousarray(np.asarray(np_inputs["Wkv2"], np.float32)).astype(ml_dtypes.bfloat16)
    return [{
        "x1T": np.ascontiguousarray(x1[b].T).astype(ml_dtypes.bfloat16),
        "x2T": np.ascontiguousarray(x2[b].T).astype(ml_dtypes.bfloat16),
        "x1N": np.ascontiguousarray(x1[b]).astype(ml_dtypes.bfloat16),
        "x2N": np.ascontiguousarray(x2[b]).astype(ml_dtypes.bfloat16),
        "Wsqkv": Wsq_b,
        "Wkv1": Wkv1_b,
        "Wkv2": Wkv2_b,
    } for b in range(B)]


def _unpack_results(results):
    o1 = np.stack([np.asarray(results[b]["o1T"]).astype(np.float32).T for b in range(B)])
    o2 = np.stack([np.asarray(results[b]["o2T"]).astype(np.float32).T for b in range(B)])
    return o1, o2


def kernel(x1, x2, Wsqkv1, Wkv1, Wkv2, num_heads=16, selfattn=1, **_unused):
    in_maps = _make_in_maps(dict(x1=x1, x2=x2, Wsqkv1=Wsqkv1, Wkv1=Wkv1, Wkv2=Wkv2))
    nc = _get_nc()
    res = run_bass_kernel_spmd(nc, in_maps, core_ids=list(range(B)),
                               trace=bool(int(os.environ.get("KERNEL_TRACE", "0"))))
    _CACHE["last_result"] = res
    return _unpack_results(res.results)
